# revision 17
# baseline (speedup 1.0000x reference)
"""CrossEntropyBoundSmoothLoss on 8 Trainium2 NeuronCores (Bass/Tile).

Math: loss*N = sum_t [ Tt_t * log(Z_t) - sum_l T[t,l]*X[t,l] ],
Z_t = sum_l exp(X[t,l])  (logits ~N(0,1): no max-subtraction needed),
T = smoothed targets (<=6 nonzeros per row).

Split: the O(N*L) work (exp + row sums) runs on device; the O(N) sparse
parts run on host: T has at most 2D+2 nonzeros per row, so the dot term
sum_l T[t,l]*X[t,l] is a handful of gathers, and the Tt-weighted
log-sum is 131k fp64 ops. Shipping dense T (3.3MB/core int8) or even a
per-row Tt vector to the device is pure DMA waste.

Device per core (16384 rows x 200 labels; rows on partitions, RP rows
per partition per tile):
  - DMA: X as fp8 e4m3 (quantization shifts the loss by ~3e-5 rel --
    gate is 2e-2), 1B/elem -> 3.28MB/core total.
  - ACT: one big exp per tile, fp8 in -> bf16 out scratch (et).
  - DVE: one segmented tensor_reduce per tile -> per-row Z in a
    persistent [128, RPC/128] f32 tile; single DMA out at the end.
ACT is the steady-state bottleneck: 25600 exp/lane @1.2GHz + ~352cy
fixed overhead per instr => ~22us/core; fp8 DMA ~11us hides under it.

Host post: Z -> fp64 log, weighted by Tt, minus sparse dot, /N.

Sharding: whole sequences per core (rows are B*S row-major; smoothing
windows stay within a sequence), host does the scalar combine.
"""

import numpy as np
import ml_dtypes

B = 64
S = 2048
L = 200
E = 0.1
D = 2
N_ROWS = B * S            # 131072
N_CORES = 8
RPC = N_ROWS // N_CORES   # 16384 rows per core
RP = 32                   # rows per partition per tile
BUFS = 4
DMA_SPLIT = 1
EXP_SPLIT = 1
XDT = "fp8"               # "fp8" | "bf16"
RED = "tree3"             # reduction: "tr" | "tree" | "tree3"
UNROLL = 8                # kernel executions per For_i iteration (loop mode)
BOUND_IDS = np.arange(0, L, 10)

_NP_XDT = {"fp8": ml_dtypes.float8_e4m3, "bf16": ml_dtypes.bfloat16}


def host_tt_dot(logits: np.ndarray, label_ids: np.ndarray):
    """Per-row target mass Tt and sparse dot sum_l T[t,l]*X[t,l]. Exact.

    Reference semantics: boundary occurrences at t' spread E/w over
    [t'-D, t'+D] (within the sequence) with 1-E at the center; where
    windows of the same label overlap, the largest t' wins. Non-boundary
    own labels add plain one-hot. All values are multiples of 1/120.
    """
    lab = label_ids.reshape(B, S).astype(np.int64)
    X3 = np.asarray(logits, np.float32).reshape(B, S, L)
    is_bound = np.zeros(L, bool)
    is_bound[BOUND_IDS] = True
    t = np.arange(S)
    offs = list(range(-D, D + 1))
    masks, labs_o, vals = [], [], []
    for o in offs:
        tp = t + o
        valid = (tp >= 0) & (tp < S)
        tpc = np.clip(tp, 0, S - 1)
        lo = lab[:, tpc]
        masks.append(valid[None, :] & is_bound[lo])
        labs_o.append(lo)
        w = np.minimum(S - 1, tpc + D) - np.maximum(0, tpc - D)
        vals.append(np.where(tp == t, 108, 12 // np.maximum(w, 1)).astype(np.float64))
    Tt = np.zeros((B, S), np.float64)
    dot = np.zeros((B, S), np.float64)
    for i in range(len(offs)):
        kill = np.zeros((B, S), bool)
        for j in range(i + 1, len(offs)):
            kill |= masks[j] & (labs_o[j] == labs_o[i])
        m = masks[i] & ~kill
        val = vals[i][None, :] / 120.0
        xg = np.take_along_axis(X3, labs_o[i][..., None], axis=2)[..., 0]
        Tt += np.where(m, val, 0.0)
        dot += np.where(m, val * xg.astype(np.float64), 0.0)
    nb = ~is_bound[lab]
    xown = np.take_along_axis(X3, lab[..., None], axis=2)[..., 0]
    Tt += nb
    dot += np.where(nb, xown.astype(np.float64), 0.0)
    return Tt.reshape(N_ROWS), dot.reshape(N_ROWS)


_NC_CACHE = {}


def _build_nc(rp: int = RP, bufs: int = BUFS, dma_split: int = DMA_SPLIT,
              exp_split: int = EXP_SPLIT, loop_n: int = 1, xdt: str = XDT,
              mode: str = "full", unroll: int = UNROLL, etspace: str = "SBUF",
              etdt: str = "bf16", zdt: str = "f32", red: str = RED):
    key = (rp, bufs, dma_split, exp_split, loop_n, xdt, mode, unroll, etspace,
           etdt, zdt, red)
    if key in _NC_CACHE:
        return _NC_CACHE[key]
    RP = rp
    NTILES = RPC // (128 * RP)
    from contextlib import ExitStack, nullcontext

    import concourse.bacc as bacc
    import concourse.mybir as mybir
    import concourse.tile as tile

    f32 = mybir.dt.float32
    bf16 = mybir.dt.bfloat16
    xdt_b = {"fp8": mybir.dt.float8e4, "bf16": bf16}[xdt]
    etdt_b = {"fp8": mybir.dt.float8e4, "bf16": bf16, "f32": f32}[etdt]
    zdt_b = {"bf16": bf16, "f32": f32}[zdt]
    nc = bacc.Bacc("TRN2", debug=False, num_devices=N_CORES)
    x_d = nc.dram_tensor("x", [RPC, L], xdt_b, kind="ExternalInput")
    z_d = nc.dram_tensor("z", [128, NTILES * RP], zdt_b, kind="ExternalOutput")

    # row r of the shard = ti*128*RP + p*RP + s  ->  z[p, ti*RP + s]
    xv = x_d.ap().rearrange("(t p s) l -> t p s l", t=NTILES, p=128, s=RP)

    with tile.TileContext(nc) as tc, ExitStack() as ctx:
        xp = ctx.enter_context(tc.tile_pool(name="xp", bufs=bufs))
        ep = ctx.enter_context(tc.tile_pool(name="ep", bufs=bufs, space=etspace))
        sp = ctx.enter_context(tc.tile_pool(name="sp", bufs=1))
        hp = (
            ctx.enter_context(tc.tile_pool(name="hp", bufs=max(2, bufs - 1)))
            if red.startswith("tree") else None
        )
        z_sb = sp.tile([128, NTILES * RP], zdt_b)

        def body():
            for ti in range(NTILES):
                xt = xp.tile([128, RP, L], xdt_b)
                if dma_split == 1:
                    nc.sync.dma_start(xt[:], xv[ti])
                else:
                    step = RP // dma_split
                    for d in range(dma_split):
                        nc.sync.dma_start(
                            xt[:, d * step : (d + 1) * step, :],
                            xv[ti][:, d * step : (d + 1) * step, :],
                        )
                et = ep.tile([128, RP, 1, L], etdt_b)
                bnds = [(RP * j) // exp_split for j in range(exp_split + 1)]
                for j in range(exp_split):
                    lo, hi = bnds[j], bnds[j + 1]
                    if mode == "noact":
                        # tiny ACT: keeps the dep chain, ~zero ACT busy
                        nc.scalar.activation(
                            et[:, lo : lo + 1, 0, :1],
                            xt[:, lo : lo + 1, :1],
                            mybir.ActivationFunctionType.Exp,
                        )
                    else:
                        nc.scalar.activation(
                            et[:, lo:hi, 0, :],
                            xt[:, lo:hi, :],
                            mybir.ActivationFunctionType.Exp,
                        )
                        if mode == "act2":
                            nc.scalar.activation(
                                et[:, lo:hi, 0, :],
                                xt[:, lo:hi, :],
                                mybir.ActivationFunctionType.Exp,
                            )
                    lp = (
                        nc.allow_low_precision(reason="z accum; error ~1e-5 rel")
                        if zdt != "f32" else nullcontext()
                    )
                    with lp:
                        if mode == "nodve":
                            nc.vector.tensor_reduce(
                                z_sb[:, ti * RP + lo : ti * RP + lo + 1],
                                et[:, lo : lo + 1, 0, :1],
                                axis=mybir.AxisListType.X,
                                op=mybir.AluOpType.add,
                            )
                        else:
                            reps = 2 if mode == "dve2" else 1
                            for _ in range(reps):
                                if red == "pool":
                                    # mean over the 200-label window; host
                                    # adds log(200) back after its log()
                                    nc.vector.pool_avg(
                                        z_sb[:, ti * RP + lo : ti * RP + hi],
                                        et[:, lo:hi, :, :],
                                    )
                                elif red.startswith("tree"):
                                    # pairwise halving: tensor_tensor bf16
                                    # runs 2 elem/cycle vs reduce's 1
                                    k = hi - lo
                                    h1 = hp.tile([128, RP, 100], bf16)
                                    nc.vector.tensor_add(
                                        h1[:, lo:hi, :],
                                        et[:, lo:hi, 0, 0:100],
                                        et[:, lo:hi, 0, 100:200],
                                    )
                                    src = h1[:, lo:hi, :]
                                    if red == "tree3":
                                        h2 = hp.tile([128, RP, 50], bf16)
                                        nc.vector.tensor_add(
                                            h2[:, lo:hi, :],
                                            h1[:, lo:hi, 0:50],
                                            h1[:, lo:hi, 50:100],
                                        )
                                        src = h2[:, lo:hi, :]
                                    nc.vector.tensor_reduce(
                                        z_sb[:, ti * RP + lo : ti * RP + hi],
                                        src,
                                        axis=mybir.AxisListType.X,
                                        op=mybir.AluOpType.add,
                                    )
                                else:
                                    nc.vector.tensor_reduce(
                                        z_sb[:, ti * RP + lo : ti * RP + hi],
                                        et[:, lo:hi, 0, :],
                                        axis=mybir.AxisListType.X,
                                        op=mybir.AluOpType.add,
                                    )

        assert loop_n == 1 or loop_n % unroll == 0
        loop_cm = (
            tc.For_i(0, loop_n // unroll, 1) if loop_n > 1 else nullcontext()
        )
        with loop_cm:
            for _ in range(unroll if loop_n > 1 else 1):
                body()
        nc.sync.dma_start(z_d.ap(), z_sb[:])

    nc.compile()
    _NC_CACHE[key] = nc
    return nc


_HOST = {}


def make_in_maps(logits: np.ndarray, label_ids: np.ndarray, rp: int = RP,
                 xdt: str = XDT):
    logits = np.ascontiguousarray(np.asarray(logits, dtype=np.float32))
    lab = np.asarray(label_ids).astype(np.int64)
    Tt, dot = host_tt_dot(logits, lab)
    _HOST["tt"] = Tt
    _HOST["dot_total"] = float(dot.sum())
    xq = logits.astype(_NP_XDT[xdt])
    return [{"x": xq[c * RPC : (c + 1) * RPC]} for c in range(N_CORES)]


def combine(results, rp: int = RP) -> np.ndarray:
    NTILES = RPC // (128 * rp)
    total = 0.0
    for c, r in enumerate(results):
        z = np.asarray(r["z"], np.float64)
        # z[p, ti*RP+s] -> row ti*128*RP + p*RP + s of this core's shard
        z_rows = z.reshape(128, NTILES, rp).transpose(1, 0, 2).reshape(RPC)
        tt = _HOST["tt"][c * RPC : (c + 1) * RPC]
        total += float(np.dot(tt, np.log(z_rows)))
    total -= _HOST["dot_total"]
    return np.asarray(np.float32(total / N_ROWS))


def kernel(logits, label_ids) -> np.ndarray:
    from concourse.bass_utils import run_bass_kernel_spmd

    nc = _build_nc()
    in_maps = make_in_maps(logits, label_ids)
    res = run_bass_kernel_spmd(nc, in_maps, core_ids=list(range(N_CORES)))
    return combine(res.results)


# revision 20
# speedup vs baseline: 1.5060x; 1.5060x over previous
"""CrossEntropyBoundSmoothLoss on 8 Trainium2 NeuronCores (Bass/Tile).

Math: loss*N = sum_t [ Tt_t * log(Z_t) - sum_l T[t,l]*X[t,l] ],
Z_t = sum_l exp(X[t,l])  (logits ~N(0,1): no max-subtraction needed),
T = smoothed targets (<=6 nonzeros per row).

Split: the O(N*L) work (exp + row sums) runs on device; the O(N) sparse
parts run on host: T has at most 2D+2 nonzeros per row, so the dot term
sum_l T[t,l]*X[t,l] is a handful of gathers, and the Tt-weighted
log-sum is 131k fp64 ops. Shipping dense T (3.3MB/core int8) or even a
per-row Tt vector to the device is pure DMA waste.

Device per core (16384 rows x 200 labels; rows on partitions, RP rows
per partition per tile):
  - DMA: X as fp8 e4m3 (quantization shifts the loss by ~3e-5 rel --
    gate is 2e-2), 1B/elem -> 3.28MB/core total.
  - ACT: one big exp per tile, fp8 in -> bf16 out scratch (et).
  - DVE: one segmented tensor_reduce per tile -> per-row Z in a
    persistent [128, RPC/128] f32 tile; single DMA out at the end.
ACT is the steady-state bottleneck: 25600 exp/lane @1.2GHz + ~352cy
fixed overhead per instr => ~22us/core; fp8 DMA ~11us hides under it.

Host post: Z -> fp64 log, weighted by Tt, minus sparse dot, /N.

Sharding: whole sequences per core (rows are B*S row-major; smoothing
windows stay within a sequence), host does the scalar combine.
"""

import numpy as np
import ml_dtypes

B = 64
S = 2048
L = 200
E = 0.1
D = 2
N_ROWS = B * S            # 131072
N_CORES = 8
RPC = N_ROWS // N_CORES   # 16384 rows per core
RP = 32                   # rows per partition per tile
BUFS = 4
DMA_SPLIT = 1
EXP_SPLIT = 1
XDT = "fp8"               # "fp8" | "bf16"
RED = "tree3"             # reduction: "tr" | "tree" | "tree3"
UNROLL = 8                # kernel executions per For_i iteration (loop mode)
BOUND_IDS = np.arange(0, L, 10)

_NP_XDT = {"fp8": ml_dtypes.float8_e4m3, "bf16": ml_dtypes.bfloat16}


def host_tt_dot(logits: np.ndarray, label_ids: np.ndarray):
    """Per-row target mass Tt and sparse dot sum_l T[t,l]*X[t,l]. Exact.

    Reference semantics: boundary occurrences at t' spread E/w over
    [t'-D, t'+D] (within the sequence) with 1-E at the center; where
    windows of the same label overlap, the largest t' wins. Non-boundary
    own labels add plain one-hot. All values are multiples of 1/120.
    """
    lab = label_ids.reshape(B, S).astype(np.int64)
    X3 = np.asarray(logits, np.float32).reshape(B, S, L)
    is_bound = np.zeros(L, bool)
    is_bound[BOUND_IDS] = True
    t = np.arange(S)
    offs = list(range(-D, D + 1))
    masks, labs_o, vals = [], [], []
    for o in offs:
        tp = t + o
        valid = (tp >= 0) & (tp < S)
        tpc = np.clip(tp, 0, S - 1)
        lo = lab[:, tpc]
        masks.append(valid[None, :] & is_bound[lo])
        labs_o.append(lo)
        w = np.minimum(S - 1, tpc + D) - np.maximum(0, tpc - D)
        vals.append(np.where(tp == t, 108, 12 // np.maximum(w, 1)).astype(np.float64))
    Tt = np.zeros((B, S), np.float64)
    dot = np.zeros((B, S), np.float64)
    for i in range(len(offs)):
        kill = np.zeros((B, S), bool)
        for j in range(i + 1, len(offs)):
            kill |= masks[j] & (labs_o[j] == labs_o[i])
        m = masks[i] & ~kill
        val = vals[i][None, :] / 120.0
        xg = np.take_along_axis(X3, labs_o[i][..., None], axis=2)[..., 0]
        Tt += np.where(m, val, 0.0)
        dot += np.where(m, val * xg.astype(np.float64), 0.0)
    nb = ~is_bound[lab]
    xown = np.take_along_axis(X3, lab[..., None], axis=2)[..., 0]
    Tt += nb
    dot += np.where(nb, xown.astype(np.float64), 0.0)
    return Tt.reshape(N_ROWS), dot.reshape(N_ROWS)


_NC_CACHE = {}


def _build_nc(rp: int = RP, bufs: int = BUFS, dma_split: int = DMA_SPLIT,
              exp_split: int = EXP_SPLIT, loop_n: int = 1, xdt: str = XDT,
              mode: str = "full", unroll: int = UNROLL, etspace: str = "SBUF",
              etdt: str = "bf16", zdt: str = "f32", red: str = RED,
              koff: int = 0):
    key = (rp, bufs, dma_split, exp_split, loop_n, xdt, mode, unroll, etspace,
           etdt, zdt, red, koff)
    if key in _NC_CACHE:
        return _NC_CACHE[key]
    RP = rp
    NTILES = RPC // (128 * RP)
    from contextlib import ExitStack, nullcontext

    import concourse.bacc as bacc
    import concourse.mybir as mybir
    import concourse.tile as tile

    f32 = mybir.dt.float32
    bf16 = mybir.dt.bfloat16
    xdt_b = {"fp8": mybir.dt.float8e4, "bf16": bf16}[xdt]
    etdt_b = {"fp8": mybir.dt.float8e4, "bf16": bf16, "f32": f32}[etdt]
    zdt_b = {"bf16": bf16, "f32": f32}[zdt]
    nc = bacc.Bacc("TRN2", debug=False, num_devices=N_CORES)
    x_d = nc.dram_tensor("x", [RPC, L], xdt_b, kind="ExternalInput")
    z_d = nc.dram_tensor("z", [128, NTILES * RP], zdt_b, kind="ExternalOutput")

    # row r of the shard = ti*128*RP + p*RP + s  ->  z[p, ti*RP + s]
    xv = x_d.ap().rearrange("(t p s) l -> t p s l", t=NTILES, p=128, s=RP)

    with tile.TileContext(nc) as tc, ExitStack() as ctx:
        xp = ctx.enter_context(tc.tile_pool(name="xp", bufs=bufs))
        ep = ctx.enter_context(tc.tile_pool(name="ep", bufs=bufs, space=etspace))
        sp = ctx.enter_context(tc.tile_pool(name="sp", bufs=1))
        hp = (
            ctx.enter_context(tc.tile_pool(name="hp", bufs=max(2, bufs - 1)))
            if red.startswith("tree") else None
        )
        ip = (
            ctx.enter_context(tc.tile_pool(name="ip", bufs=max(2, bufs - 1)))
            if koff > 0 else None
        )
        z_sb = sp.tile([128, NTILES * RP], zdt_b)
        # Schraudolph int-exp constants: exp(x) ~= bitcast_f32(A*x + B).
        # +0.5 makes a truncating f32->i32 convert behave like rounding
        # (the argument is always ~1e9 > 0); harmless if the HW rounds.
        SCH_A = 12102203.161561485   # 2^23 / ln 2
        SCH_B = 1064866805.0 + 0.5   # 127*2^23 - 486411

        def body():
            for ti in range(NTILES):
                xt = xp.tile([128, RP, L], xdt_b)
                if dma_split == 1:
                    nc.sync.dma_start(xt[:], xv[ti])
                else:
                    step = RP // dma_split
                    for d in range(dma_split):
                        nc.sync.dma_start(
                            xt[:, d * step : (d + 1) * step, :],
                            xv[ti][:, d * step : (d + 1) * step, :],
                        )
                if koff > 0:
                    # Schraudolph slabs: DVE int-exp + reduce, no ACT at all.
                    # Error ~+-3% per element; averaged over a row's 200
                    # labels and 131k rows it shifts the loss by <1e-3 even
                    # at koff=RP (gate is 2e-2).
                    it = ip.tile([128, koff, L], mybir.dt.int32)
                    nc.vector.tensor_scalar(
                        out=it[:],
                        in0=xt[:, 0:koff, :],
                        scalar1=SCH_A,
                        scalar2=SCH_B,
                        op0=mybir.AluOpType.mult,
                        op1=mybir.AluOpType.add,
                    )
                    nc.vector.tensor_reduce(
                        z_sb[:, ti * RP : ti * RP + koff],
                        it[:].bitcast(f32),
                        axis=mybir.AxisListType.X,
                        op=mybir.AluOpType.add,
                    )
                et = ep.tile([128, RP, 1, L], etdt_b)
                bnds = [koff + ((RP - koff) * j) // exp_split
                        for j in range(exp_split + 1)]
                for j in range(exp_split):
                    lo, hi = bnds[j], bnds[j + 1]
                    if mode == "noact":
                        # tiny ACT: keeps the dep chain, ~zero ACT busy
                        nc.scalar.activation(
                            et[:, lo : lo + 1, 0, :1],
                            xt[:, lo : lo + 1, :1],
                            mybir.ActivationFunctionType.Exp,
                        )
                    else:
                        nc.scalar.activation(
                            et[:, lo:hi, 0, :],
                            xt[:, lo:hi, :],
                            mybir.ActivationFunctionType.Exp,
                        )
                        if mode == "act2":
                            nc.scalar.activation(
                                et[:, lo:hi, 0, :],
                                xt[:, lo:hi, :],
                                mybir.ActivationFunctionType.Exp,
                            )
                    lp = (
                        nc.allow_low_precision(reason="z accum; error ~1e-5 rel")
                        if zdt != "f32" else nullcontext()
                    )
                    with lp:
                        if mode == "nodve":
                            nc.vector.tensor_reduce(
                                z_sb[:, ti * RP + lo : ti * RP + lo + 1],
                                et[:, lo : lo + 1, 0, :1],
                                axis=mybir.AxisListType.X,
                                op=mybir.AluOpType.add,
                            )
                        else:
                            reps = 2 if mode == "dve2" else 1
                            for _ in range(reps):
                                if red == "pool":
                                    # mean over the 200-label window; host
                                    # adds log(200) back after its log()
                                    nc.vector.pool_avg(
                                        z_sb[:, ti * RP + lo : ti * RP + hi],
                                        et[:, lo:hi, :, :],
                                    )
                                elif red.startswith("tree"):
                                    # pairwise halving: tensor_tensor bf16
                                    # runs 2 elem/cycle vs reduce's 1
                                    k = hi - lo
                                    h1 = hp.tile([128, RP, 100], bf16)
                                    nc.vector.tensor_add(
                                        h1[:, lo:hi, :],
                                        et[:, lo:hi, 0, 0:100],
                                        et[:, lo:hi, 0, 100:200],
                                    )
                                    src = h1[:, lo:hi, :]
                                    if red == "tree3":
                                        h2 = hp.tile([128, RP, 50], bf16)
                                        nc.vector.tensor_add(
                                            h2[:, lo:hi, :],
                                            h1[:, lo:hi, 0:50],
                                            h1[:, lo:hi, 50:100],
                                        )
                                        src = h2[:, lo:hi, :]
                                    nc.vector.tensor_reduce(
                                        z_sb[:, ti * RP + lo : ti * RP + hi],
                                        src,
                                        axis=mybir.AxisListType.X,
                                        op=mybir.AluOpType.add,
                                    )
                                else:
                                    nc.vector.tensor_reduce(
                                        z_sb[:, ti * RP + lo : ti * RP + hi],
                                        et[:, lo:hi, 0, :],
                                        axis=mybir.AxisListType.X,
                                        op=mybir.AluOpType.add,
                                    )

        assert loop_n == 1 or loop_n % unroll == 0
        loop_cm = (
            tc.For_i(0, loop_n // unroll, 1) if loop_n > 1 else nullcontext()
        )
        with loop_cm:
            for _ in range(unroll if loop_n > 1 else 1):
                body()
        nc.sync.dma_start(z_d.ap(), z_sb[:])

    nc.compile()
    _NC_CACHE[key] = nc
    return nc


_HOST = {}


def make_in_maps(logits: np.ndarray, label_ids: np.ndarray, rp: int = RP,
                 xdt: str = XDT):
    logits = np.ascontiguousarray(np.asarray(logits, dtype=np.float32))
    lab = np.asarray(label_ids).astype(np.int64)
    Tt, dot = host_tt_dot(logits, lab)
    _HOST["tt"] = Tt
    _HOST["dot_total"] = float(dot.sum())
    xq = logits.astype(_NP_XDT[xdt])
    return [{"x": xq[c * RPC : (c + 1) * RPC]} for c in range(N_CORES)]


def combine(results, rp: int = RP) -> np.ndarray:
    NTILES = RPC // (128 * rp)
    total = 0.0
    for c, r in enumerate(results):
        z = np.asarray(r["z"], np.float64)
        # z[p, ti*RP+s] -> row ti*128*RP + p*RP + s of this core's shard
        z_rows = z.reshape(128, NTILES, rp).transpose(1, 0, 2).reshape(RPC)
        tt = _HOST["tt"][c * RPC : (c + 1) * RPC]
        total += float(np.dot(tt, np.log(z_rows)))
    total -= _HOST["dot_total"]
    return np.asarray(np.float32(total / N_ROWS))


def kernel(logits, label_ids) -> np.ndarray:
    from concourse.bass_utils import run_bass_kernel_spmd

    nc = _build_nc()
    in_maps = make_in_maps(logits, label_ids)
    res = run_bass_kernel_spmd(nc, in_maps, core_ids=list(range(N_CORES)))
    return combine(res.results)
